# revision 34
# baseline (speedup 1.0000x reference)
"""Depthwise symmetric 7x7 Conv2d (all channels share one kernel) on 8 trn2 cores.

Strategy: the conv is 2048 independent 224x224 planes convolved with a single
7x7 filter. Shard planes across 8 cores (256 planes/core). On each core,
compute the conv as 7 accumulating TensorE matmuls per output tile:
  - H-convolution via a banded (7-diagonal) stationary matrix lhsT [115, 112]
  - W-shift per tap applied by sliding the moving operand's free-dim window,
    accumulating into the same PSUM tile (edge taps use clipped column ranges,
    so no zero-padding or memsets are needed anywhere)
The 25 kernel params arrive as a runtime input, so the banded matrices are
assembled host-side in numpy (microseconds) and DMA'd once per core (~720 KB).

Matmul operands are bitcast to float32r (TF32-rate path: 1 col/cycle at
N>=256 vs 4 col/cycle for plain fp32); accumulation is fp32 in PSUM.

The kernel is TensorE-streaming-bound (7 taps x 448 cols x 256 tile-groups
~= 335us/core at 2.4GHz vs ~290us of HBM traffic), so the schedule exists to
keep the PE issue stream gapless:
  - input DMAs batched 8 planes per transfer on the SP HWDGE queue (cuts
    HWDGE serialization 4x vs per-pair transfers)
  - output DMAs on the gpsimd SWDGE queue so they can't head-of-line block
    input prefetch; PSUM evacuated by ScalarE into 8-plane staging tiles
  - dummy warm-up matmuls during the initial DMA wait so the PE clock (HAM
    throttle) is at full rate when real work starts; first input + band
    transfers split/ordered to minimize time-to-first-matmul
  - per-pair tail DMAs so the final drain is one small transfer
"""

import numpy as np

import concourse.bacc as bacc
import concourse.bass as bass
import concourse.mybir as mybir
from concourse import tile
from concourse.bass_utils import run_bass_kernel_spmd

KS = 7          # kernel size
PAD = 3         # same padding
H = W = 224
N_BATCH = 16
CN = 128
N_CORES = 8
N_PLANES = N_BATCH * CN                  # 2048
PLANES_PER_CORE = N_PLANES // N_CORES    # 256
MT = 112        # output rows per H-tile (2 tiles cover 224)
KT = MT + PAD   # 115 input rows per H-tile (halo clipped at image edges)
PP = 2          # planes processed per matmul group (N = PP*224 = 448 <= 512)
WP = W + 2 * PAD  # padded plane width in SBUF (230)

MAXNUM = (KS * KS + KS % 2) // 2  # 25

F32 = mybir.dt.float32
F32R = mybir.dt.float32r


def _sym_weight(kv: np.ndarray) -> np.ndarray:
    """Reproduce the reference's 180-deg symmetric 7x7 kernel assembly."""
    flat = np.zeros(KS * KS, np.float32)
    idx = np.arange(MAXNUM)
    flat[idx] = kv
    flat[KS * KS - 1 - idx] = kv
    return flat.reshape(KS, KS)


def _banded_packed(k2d: np.ndarray) -> np.ndarray:
    """Banded H-conv matrices, packed [KT, 2(tile variant), 7(dx), MT].

    Variant 0 (top H-tile): input rows 0..115, output rows 0..112
        B[p, m] = k2d[p - m + 3, dx]  (band clipped at the top edge)
    Variant 1 (bottom H-tile): input rows 109..224, output rows 112..224
        B[p, m] = k2d[p - m, dx]      (band clipped at the bottom edge)
    """
    p = np.arange(KT)[:, None]
    m = np.arange(MT)[None, :]
    out = np.zeros((KT, 2, KS, MT), np.float32)
    for var, off in ((0, 3), (1, 0)):
        dy = p - m + off
        valid = (dy >= 0) & (dy < KS)
        dyc = np.clip(dy, 0, KS - 1)
        for dx in range(KS):
            out[:, var, dx, :] = np.where(valid, k2d[dyc, dx], 0.0)
    return np.ascontiguousarray(out)


PB = 8          # planes per DMA block (4 matmul sub-groups of PP=2)


def _build_nc(planes_per_core: int) -> bass.Bass:
    nc = bacc.Bacc(
        "TRN2", target_bir_lowering=False, debug=False, num_devices=N_CORES
    )
    x = nc.dram_tensor("x", [planes_per_core, H, W], F32R, kind="ExternalInput")
    b = nc.dram_tensor("b", [KT, 2, KS, MT], F32R, kind="ExternalInput")
    y = nc.dram_tensor("y", [planes_per_core, H, W], F32, kind="ExternalOutput")

    n_blocks = planes_per_core // PB
    XBUFS = 6
    YBUFS = 6
    N_WARMUP = 9

    with tile.TileContext(nc) as tc:
        with (
            tc.tile_pool(name="bpool", bufs=1) as bpool,
            tc.tile_pool(name="xpool", bufs=XBUFS) as xpool,
            tc.tile_pool(name="ppool", bufs=8, space="PSUM") as ppool,
            tc.tile_pool(name="ypool", bufs=YBUFS) as ypool,
        ):
            # Band upload, split per tile-variant on the Activation HWDGE
            # queue in first-use order: the SP queue stays free for the first
            # input chunks, and the t=0 matrices land in ~4us instead of
            # serializing a 721KB transfer ahead of the input prefetch.
            bsb = bpool.tile([KT, 2, KS, MT], F32R)
            nc.scalar.dma_start(bsb[:, 0, :, :], b[:, 0, :, :])
            nc.scalar.dma_start(bsb[:, 1, :, :], b[:, 1, :, :])

            # PE warm-up: the PE clock is throttled (HAM) until it has been
            # continuously busy for ~3.4us, and the first input DMA takes
            # ~5us to land. Chew through dummy matmuls on a zeroed scratch
            # tile during that window so the first real matmul runs at full
            # clock. Results go to the first psum ring slot and are never
            # read.
            wt = bpool.tile([KT, W], F32R)
            nc.vector.memset(wt[:].bitcast(F32), 0.0)
            wp = ppool.tile([MT, PP, W], F32, tag="pt")
            for _ in range(N_WARMUP):
                nc.tensor.matmul(wp[:, 0, :], wt[:, 0:MT], wt[:], start=True, stop=True)

            # Per-plane layout [pad3 | 224 | pad3] so every tap is a
            # full-width matmul (fp32r needs even N and even PSUM offset,
            # which clipped edge taps would violate). The DMA only ever
            # writes the interior [3:227], so the pad columns are zeroed
            # once per ring buffer here instead of once per iteration.
            # (These memsets queue on DVE behind the warm-up tile's memset,
            # which must run first.)
            xbufs = []
            for _ in range(XBUFS):
                xt = xpool.tile([KT, PB, WP], F32R, tag="xt")
                nc.vector.memset(xt[:, :, 0:PAD].bitcast(F32), 0.0)
                nc.vector.memset(xt[:, :, PAD + W : WP].bitcast(F32), 0.0)
                xbufs.append(xt)

            n_iters = n_blocks * 2
            it = 0
            for g in range(n_blocks):
                for t in range(2):
                    r0 = 0 if t == 0 else H - KT
                    xt = xbufs[it % XBUFS]
                    if it == 0:
                        # Split the very first load into per-pair chunks so
                        # the first matmul group starts after ~1/4 of the
                        # transfer instead of waiting for the whole block.
                        for k in range(PB // PP):
                            nc.sync.dma_start(
                                xt[:, PP * k : PP * (k + 1), PAD : PAD + W],
                                x[PP * k : PP * (k + 1), r0 : r0 + KT, :].transpose(
                                    [1, 0, 2]
                                ),
                            )
                    else:
                        nc.sync.dma_start(
                            xt[:, :, PAD : PAD + W],
                            x[PB * g : PB * g + PB, r0 : r0 + KT, :].transpose(
                                [1, 0, 2]
                            ),
                        )
                    yt = ypool.tile([MT, PB, W], F32, tag="yt")
                    for k in range(PB // PP):
                        pt = ppool.tile([MT, PP, W], F32, tag="pt")
                        for dx in range(KS):
                            nc.tensor.matmul(
                                pt[:],
                                bsb[:, t, dx, :],
                                xt[:, PP * k : PP * (k + 1), dx : dx + W],
                                start=(dx == 0),
                                stop=(dx == KS - 1),
                            )
                        nc.scalar.copy(yt[:, PP * k : PP * (k + 1), :], pt[:])
                        if it == n_iters - 1:
                            # Tail: per-pair out-DMAs on the (by now idle) SP
                            # HWDGE queue right after each copy, so the final
                            # drain is one small transfer, not a full block.
                            nc.sync.dma_start(
                                y[
                                    PB * g + PP * k : PB * g + PP * (k + 1),
                                    MT * t : MT * (t + 1),
                                    :,
                                ].transpose([1, 0, 2]),
                                yt[:, PP * k : PP * (k + 1), :],
                            )
                    if it != n_iters - 1:
                        # Out-DMAs go on the (otherwise idle) gpsimd queue so
                        # an out-DMA waiting on its copies can't head-of-line
                        # block the next input prefetch on the SP queue.
                        nc.gpsimd.dma_start(
                            y[PB * g : PB * g + PB, MT * t : MT * (t + 1), :].transpose(
                                [1, 0, 2]
                            ),
                            yt[:],
                        )
                    it += 1
    nc.compile()
    return nc


_NC_CACHE: dict[int, bass.Bass] = {}


def _get_nc(planes_per_core: int) -> bass.Bass:
    if planes_per_core not in _NC_CACHE:
        _NC_CACHE[planes_per_core] = _build_nc(planes_per_core)
    return _NC_CACHE[planes_per_core]


def _run(x_planes: np.ndarray, kv: np.ndarray, **spmd_kwargs):
    """x_planes: [n_planes, 224, 224] fp32; returns (out_planes, BassKernelResults)."""
    n_planes = x_planes.shape[0]
    per_core = n_planes // N_CORES
    assert per_core * N_CORES == n_planes and per_core % PB == 0
    k2d = _sym_weight(np.asarray(kv, np.float32))
    bnp = _banded_packed(k2d)
    nc = _get_nc(per_core)
    in_maps = [
        {"x": np.ascontiguousarray(x_planes[i * per_core : (i + 1) * per_core]),
         "b": bnp}
        for i in range(N_CORES)
    ]
    res = run_bass_kernel_spmd(nc, in_maps, core_ids=list(range(N_CORES)), **spmd_kwargs)
    out = np.concatenate([r["y"] for r in res.results], axis=0)
    return out, res


def kernel(x: np.ndarray, kv: np.ndarray) -> np.ndarray:
    x = np.ascontiguousarray(np.asarray(x, np.float32))
    planes = x.reshape(N_PLANES, H, W)
    out, _ = _run(planes, kv)
    return out.reshape(N_BATCH, CN, H, W)



# revision 43
# speedup vs baseline: 1.0073x; 1.0073x over previous
"""Depthwise symmetric 7x7 Conv2d (all channels share one kernel) on 8 trn2 cores.

Strategy: the conv is 2048 independent 224x224 planes convolved with a single
7x7 filter. Shard planes across 8 cores (256 planes/core). On each core,
compute the conv as 7 accumulating TensorE matmuls per output tile:
  - H-convolution via a banded (7-diagonal) stationary matrix lhsT [115, 112]
  - W-shift per tap applied by sliding the moving operand's free-dim window,
    accumulating into the same PSUM tile (edge taps use clipped column ranges,
    so no zero-padding or memsets are needed anywhere)
The 25 kernel params arrive as a runtime input, so the banded matrices are
assembled host-side in numpy (microseconds) and DMA'd once per core (~720 KB).

Matmul operands are bitcast to float32r (TF32-rate path: 1 col/cycle at
N>=256 vs 4 col/cycle for plain fp32); accumulation is fp32 in PSUM.

The kernel is TensorE-streaming-bound (7 taps x 448 cols x 256 tile-groups
~= 335us/core at 2.4GHz vs ~290us of HBM traffic), so the schedule exists to
keep the PE issue stream gapless:
  - input DMAs batched 8 planes per transfer on the SP HWDGE queue (cuts
    HWDGE serialization 4x vs per-pair transfers)
  - output DMAs on the gpsimd SWDGE queue so they can't head-of-line block
    input prefetch; PSUM evacuated by ScalarE into 8-plane staging tiles
  - dummy warm-up matmuls during the initial DMA wait so the PE clock (HAM
    throttle) is at full rate when real work starts; first input + band
    transfers split/ordered to minimize time-to-first-matmul
  - per-pair tail DMAs so the final drain is one small transfer
"""

import numpy as np

import concourse.bacc as bacc
import concourse.bass as bass
import concourse.mybir as mybir
from concourse import tile
from concourse.bass_utils import run_bass_kernel_spmd

KS = 7          # kernel size
PAD = 3         # same padding
H = W = 224
N_BATCH = 16
CN = 128
N_CORES = 8
N_PLANES = N_BATCH * CN                  # 2048
PLANES_PER_CORE = N_PLANES // N_CORES    # 256
MT = 112        # output rows per H-tile (2 tiles cover 224)
KT = MT + PAD   # 115 input rows per H-tile (halo clipped at image edges)
PP = 2          # planes processed per matmul group (N = PP*224 = 448 <= 512)
WP = W + 2 * PAD  # padded plane width in SBUF (230)

MAXNUM = (KS * KS + KS % 2) // 2  # 25

F32 = mybir.dt.float32
F32R = mybir.dt.float32r


def _sym_weight(kv: np.ndarray) -> np.ndarray:
    """Reproduce the reference's 180-deg symmetric 7x7 kernel assembly."""
    flat = np.zeros(KS * KS, np.float32)
    idx = np.arange(MAXNUM)
    flat[idx] = kv
    flat[KS * KS - 1 - idx] = kv
    return flat.reshape(KS, KS)


def _banded_packed(k2d: np.ndarray) -> np.ndarray:
    """Banded H-conv matrices, packed [KT, 2(tile variant), 7(dx), MT].

    Variant 0 (top H-tile): input rows 0..115, output rows 0..112
        B[p, m] = k2d[p - m + 3, dx]  (band clipped at the top edge)
    Variant 1 (bottom H-tile): input rows 109..224, output rows 112..224
        B[p, m] = k2d[p - m, dx]      (band clipped at the bottom edge)
    """
    p = np.arange(KT)[:, None]
    m = np.arange(MT)[None, :]
    out = np.zeros((KT, 2, KS, MT), np.float32)
    for var, off in ((0, 3), (1, 0)):
        dy = p - m + off
        valid = (dy >= 0) & (dy < KS)
        dyc = np.clip(dy, 0, KS - 1)
        for dx in range(KS):
            out[:, var, dx, :] = np.where(valid, k2d[dyc, dx], 0.0)
    return np.ascontiguousarray(out)


PB = 8          # planes per DMA block (4 matmul sub-groups of PP=2)

# Tap schedule: (dx, lo, hi) = horizontal tap and its out-column range.
# Center tap (dx=3) first at full width; edge taps trimmed by the number of
# output columns that only ever receive zero-pad contributions, rounded down
# to even so fp32r PSUM offsets/sizes stay even.
TAPS = [
    (3, 0, W),
    (0, 2, W),
    (1, 2, W),
    (2, 0, W),
    (4, 0, W),
    (5, 0, W - 2),
    (6, 0, W - 2),
]


def _build_nc(planes_per_core: int) -> bass.Bass:
    nc = bacc.Bacc(
        "TRN2", target_bir_lowering=False, debug=False, num_devices=N_CORES
    )
    x = nc.dram_tensor("x", [planes_per_core, H, W], F32R, kind="ExternalInput")
    b = nc.dram_tensor("b", [KT, 2, KS, MT], F32R, kind="ExternalInput")
    y = nc.dram_tensor("y", [planes_per_core, H, W], F32, kind="ExternalOutput")

    n_blocks = planes_per_core // PB
    XBUFS = 6
    YBUFS = 6
    N_WARMUP = 9

    with tile.TileContext(nc) as tc:
        with (
            tc.tile_pool(name="bpool", bufs=1) as bpool,
            tc.tile_pool(name="xpool", bufs=XBUFS) as xpool,
            tc.tile_pool(name="ppool", bufs=8, space="PSUM") as ppool,
            tc.tile_pool(name="ypool", bufs=YBUFS) as ypool,
        ):
            # Band upload, split per tile-variant on the Activation HWDGE
            # queue in first-use order: the SP queue stays free for the first
            # input chunks, and the t=0 matrices land in ~4us instead of
            # serializing a 721KB transfer ahead of the input prefetch.
            # The band rides the gpsimd SWDGE path, which does not touch the
            # global HWDGE device, so the first input chunk gets HWDGE slot 1
            # and the band transfer runs fully in parallel with it.
            # The t=1 half is deferred until after the first input block so
            # its transfer can't queue ahead of input chunk 0 on the DMA
            # engines (it isn't needed until the second iteration, ~8us in).
            bsb = bpool.tile([KT, 2, KS, MT], F32R)
            nc.gpsimd.dma_start(bsb[:, 0, :, :], b[:, 0, :, :])

            # PE warm-up: the PE clock is throttled (HAM) until it has been
            # continuously busy for ~3.4us, and the first input DMA takes
            # ~5us to land. Chew through dummy matmuls on a zeroed scratch
            # tile during that window so the first real matmul runs at full
            # clock. Results go to the first psum ring slot and are never
            # read.
            wt = bpool.tile([KT, W], F32R)
            nc.vector.memset(wt[:].bitcast(F32), 0.0)
            wp = ppool.tile([MT, PP, W], F32, tag="pt")
            for _ in range(N_WARMUP):
                nc.tensor.matmul(wp[:, 0, :], wt[:, 0:MT], wt[:], start=True, stop=True)

            # Per-plane layout [pad3 | 224 | pad3] so every tap is a
            # full-width matmul (fp32r needs even N and even PSUM offset,
            # which clipped edge taps would violate). The DMA only ever
            # writes the interior [3:227], so the pad columns are zeroed
            # once per ring buffer here instead of once per iteration.
            # (These memsets queue on DVE behind the warm-up tile's memset,
            # which must run first.)
            xbufs = []
            for _ in range(XBUFS):
                xt = xpool.tile([KT, PB, WP], F32R, tag="xt")
                nc.vector.memset(xt[:, :, 0:PAD].bitcast(F32), 0.0)
                nc.vector.memset(xt[:, :, PAD + W : WP].bitcast(F32), 0.0)
                xbufs.append(xt)

            n_iters = n_blocks * 2
            it = 0
            for g in range(n_blocks):
                for t in range(2):
                    r0 = 0 if t == 0 else H - KT
                    xt = xbufs[it % XBUFS]
                    if it == 0:
                        # Split the very first load into per-pair chunks so
                        # the first matmul group starts after ~1/4 of the
                        # transfer instead of waiting for the whole block.
                        for k in range(PB // PP):
                            nc.sync.dma_start(
                                xt[:, PP * k : PP * (k + 1), PAD : PAD + W],
                                x[PP * k : PP * (k + 1), r0 : r0 + KT, :].transpose(
                                    [1, 0, 2]
                                ),
                            )
                    else:
                        nc.sync.dma_start(
                            xt[:, :, PAD : PAD + W],
                            x[PB * g : PB * g + PB, r0 : r0 + KT, :].transpose(
                                [1, 0, 2]
                            ),
                        )
                    if it == 1:
                        nc.gpsimd.dma_start(bsb[:, 1, :, :], b[:, 1, :, :])
                    yt = ypool.tile([MT, PB, W], F32, tag="yt")
                    for k in range(PB // PP):
                        pt = ppool.tile([MT, PP, W], F32, tag="pt")
                        # Edge taps only deposit zeros (from the pad columns)
                        # into the outermost output columns, so those columns
                        # are trimmed from the stream — in even-sized chunks
                        # to keep fp32r PSUM offsets even. The full-width
                        # center tap runs first (start=True) so every PSUM
                        # column is initialized before the trimmed taps
                        # accumulate.
                        for dx, lo, hi in TAPS:
                            nc.tensor.matmul(
                                pt[:, :, lo:hi],
                                bsb[:, t, dx, :],
                                xt[:, PP * k : PP * (k + 1), dx + lo : dx + hi],
                                start=(dx == PAD),
                                stop=(dx == KS - 1),
                            )
                        nc.scalar.copy(yt[:, PP * k : PP * (k + 1), :], pt[:])
                        if it == n_iters - 1:
                            # Tail: per-pair out-DMAs on the (by now idle) SP
                            # HWDGE queue right after each copy, so the final
                            # drain is one small transfer, not a full block.
                            nc.sync.dma_start(
                                y[
                                    PB * g + PP * k : PB * g + PP * (k + 1),
                                    MT * t : MT * (t + 1),
                                    :,
                                ].transpose([1, 0, 2]),
                                yt[:, PP * k : PP * (k + 1), :],
                            )
                    if it != n_iters - 1:
                        # Out-DMAs go on the (otherwise idle) gpsimd queue so
                        # an out-DMA waiting on its copies can't head-of-line
                        # block the next input prefetch on the SP queue.
                        nc.gpsimd.dma_start(
                            y[PB * g : PB * g + PB, MT * t : MT * (t + 1), :].transpose(
                                [1, 0, 2]
                            ),
                            yt[:],
                        )
                    it += 1
    nc.compile()
    return nc


_NC_CACHE: dict[int, bass.Bass] = {}


def _get_nc(planes_per_core: int) -> bass.Bass:
    if planes_per_core not in _NC_CACHE:
        _NC_CACHE[planes_per_core] = _build_nc(planes_per_core)
    return _NC_CACHE[planes_per_core]


def _run(x_planes: np.ndarray, kv: np.ndarray, **spmd_kwargs):
    """x_planes: [n_planes, 224, 224] fp32; returns (out_planes, BassKernelResults)."""
    n_planes = x_planes.shape[0]
    per_core = n_planes // N_CORES
    assert per_core * N_CORES == n_planes and per_core % PB == 0
    k2d = _sym_weight(np.asarray(kv, np.float32))
    bnp = _banded_packed(k2d)
    nc = _get_nc(per_core)
    in_maps = [
        {"x": np.ascontiguousarray(x_planes[i * per_core : (i + 1) * per_core]),
         "b": bnp}
        for i in range(N_CORES)
    ]
    res = run_bass_kernel_spmd(nc, in_maps, core_ids=list(range(N_CORES)), **spmd_kwargs)
    out = np.concatenate([r["y"] for r in res.results], axis=0)
    return out, res


def kernel(x: np.ndarray, kv: np.ndarray) -> np.ndarray:
    x = np.ascontiguousarray(np.asarray(x, np.float32))
    planes = x.reshape(N_PLANES, H, W)
    out, _ = _run(planes, kv)
    return out.reshape(N_BATCH, CN, H, W)

